# revision 39
# baseline (speedup 1.0000x reference)
"""Trainium2 Bass kernel for sparse_attention (nn_Attention_171798692167).

B=128, N=2048, DM=DQ=DA=512.  Data-parallel over 8 NeuronCores: 16 batch
rows per core, Wm/Wq/v replicated.  Per row b:
    tq = query[b] @ Wq
    e = tanh(MV[b] @ Wm + tq)          (2048, 512)
    logits_raw = e @ v                 (2048,)
    logits = logits_raw + (mask-1)*1e9
    weights = softmax(logits)
    context = weights @ MV[b]          (512,)
Returns (weights, context, logits) full-shape.
"""

import sys

import numpy as np

if "/opt/trn_rl_repo" not in sys.path:
    sys.path.insert(0, "/opt/trn_rl_repo")

NCORES = 8
B, N, D = 128, 2048, 512
BB = B // NCORES  # 16 batch rows per core
C = D // 128      # 4 chunks of the 512 feature dims
NT = N // 128     # 16 n-chunks of 128
NBK = N // 512    # 4 n-blocks of 512
NEG = -1.0e9

# tuning knobs
NB_BUFS = 6
MT_BUFS = 2
ET_BUFS = 6

_STATE: dict = {}


def _emit(ctx, tc, nc, aps):
    import concourse.bass as bass
    from concourse import masks, mybir

    F32 = mybir.dt.float32
    BF16 = mybir.dt.bfloat16
    AF = mybir.ActivationFunctionType
    ALU = mybir.AluOpType
    AX = mybir.AxisListType
    ts = bass.ts

    mv, mk, q, wm, wq, vv, o_w, o_c, o_l = aps

    const_pool = ctx.enter_context(tc.tile_pool(name="const", bufs=1))
    nb_pool = ctx.enter_context(tc.tile_pool(name="nb", bufs=NB_BUFS))
    mt_pool = ctx.enter_context(tc.tile_pool(name="mt", bufs=MT_BUFS))
    et_pool = ctx.enter_context(tc.tile_pool(name="et", bufs=ET_BUFS))
    sm_pool = ctx.enter_context(tc.tile_pool(name="sm", bufs=2))
    sm1_pool = ctx.enter_context(tc.tile_pool(name="sm1", bufs=1))
    ps_t = ctx.enter_context(tc.tile_pool(name="ps_t", bufs=2, space="PSUM"))
    ps_e = ctx.enter_context(tc.tile_pool(name="ps_e", bufs=4, space="PSUM"))
    ps_v = ctx.enter_context(tc.tile_pool(name="ps_v", bufs=1, space="PSUM"))
    ps_s = ctx.enter_context(tc.tile_pool(name="ps_s", bufs=1, space="PSUM"))

    # ---- constants / params ----
    ident = const_pool.tile([128, 128], BF16, tag="ident")
    masks.make_identity(nc, ident[:])
    identf = const_pool.tile([4, 4], F32, tag="identf")
    masks.make_identity(nc, identf[:])

    # small params first so the q^T transposes can start immediately
    qb = const_pool.tile([BB, D], BF16, tag="qb")
    nc.gpsimd.dma_start(qb[:], q[:])
    vb = const_pool.tile([128, C], BF16, tag="vb")
    nc.gpsimd.dma_start(vb[:], vv[:].rearrange("(c p) o -> p (c o)", p=128))
    # Wm, Wq as (128, c, a) bf16: lhsT chunk = WmB[:, c, ts(ac,128)]
    wmb = const_pool.tile([128, C, D], BF16, tag="wmb")
    nc.gpsimd.dma_start(wmb[:], wm[:].rearrange("(c p) a -> p c a", p=128))
    # first batch row's data before Wq: the Wq-dependent tq matmuls are not
    # needed until the first tanh, but the transposes need nb[0] immediately
    nb_first = nb_pool.tile([128, NT, D], BF16, tag="nb")
    for t in range(NT):
        nc.gpsimd.dma_start(nb_first[:, t, :], mv[0][128 * t : 128 * (t + 1), :])
    wqb = const_pool.tile([128, C, D], BF16, tag="wqb")
    nc.gpsimd.dma_start(wqb[:], wq[:].rearrange("(c p) a -> p c a", p=128))
    # suppress tiles are built per group of 4 rows (mask*1e9 - 1e9, exact
    # 0 / -1e9); mask rows land at partition offset 0 via direct row DMA

    # ---- q^T then tq^T = Wq^T q^T ----
    qtp = ps_s.tile([128, C, BB], BF16, tag="small")
    for c in range(C):
        nc.tensor.transpose(qtp[:, c, :], qb[:, ts(c, 128)], ident[0:BB, 0:BB])
    qt = const_pool.tile([128, C, BB], BF16, tag="qt")
    nc.vector.tensor_copy(qt[:], qtp[:])
    tq = const_pool.tile([128, C, BB], F32, tag="tq")
    for ac in range(C):
        tqp = ps_s.tile([128, BB], F32, tag="small")
        for c in range(C):
            nc.tensor.matmul(
                tqp[:], wqb[:, c, ts(ac, 128)], qt[:, c, :],
                start=(c == 0), stop=(c == C - 1),
            )
        nc.vector.tensor_copy(tq[:, ac, :], tqp[:])

    # ---- main loop over batch rows ----
    lr_g = None
    nb_tiles = {}
    for b in range(BB):
        g, j = b // 4, b % 4

        # natural bf16 tiles: (128 n-part, t, d); SWDGE casts f32->bf16.
        # b=0 loads per-chunk so the first transposes start early.
        if b == 0:
            nb_b = nb_first
        else:
            nb_b = nb_pool.tile([128, NT, D], BF16, tag="nb")
            nc.gpsimd.dma_start(nb_b[:], mv[b].rearrange("(t p) d -> p t d", p=128))
        nb_tiles[b] = nb_b

        if j == 0:
            lr_g = sm_pool.tile([4, N], F32, tag="lr")
            sup_g = sm1_pool.tile([4, N], F32, tag="sup")
            nc.sync.dma_start(sup_g[:], mk[4 * g : 4 * g + 4, :])
            nc.vector.tensor_scalar(sup_g[:], sup_g[:], -NEG, NEG, ALU.mult, ALU.add)

        # transpose to (128 d-part, c, n); two n-chunks per psum tile so the
        # DVE drain copies are half as many and outrun the transpose stream
        mt_b = mt_pool.tile([128, C, N], BF16, tag="mt")
        lgg = ps_v.tile([128, 512], F32, tag="lg")
        for th in range(NT // 2):
            tp = ps_t.tile([128, C, 2, 128], BF16, tag="tp")
            for c in range(C):
                for k in range(2):
                    nc.tensor.transpose(
                        tp[:, c, k, :], nb_b[:, 2 * th + k, ts(c, 128)], ident[:]
                    )
            nc.vector.tensor_copy(
                mt_b[:, :, 256 * th : 256 * th + 256],
                tp[:].rearrange("p c k n -> p c (k n)"),
            )
            if th in (2, 5):
                # ~50ns normal-mode matmul: keeps the HAM clock gate warm
                # through the transpose phase (junk value, overwritten later)
                nc.tensor.matmul(
                    lgg[0:1, 0:1], vb[:, 0:1], ident[:, 0:1],
                    start=True, stop=True,
                )
        et_ts = []
        for nb in range(NBK):
            et_t = et_pool.tile([128, C, 512], BF16, tag="et")
            et_ts.append(et_t)
            for ac in range(C):
                ep = ps_e.tile([128, 512], F32, tag="pe")
                for c in range(C):
                    nc.tensor.matmul(
                        ep[:], wmb[:, c, ts(ac, 128)], mt_b[:, c, ts(nb, 512)],
                        start=(c == 0), stop=(c == C - 1),
                    )
                # tanh(E^T + tq^T) fused on ACT; bias per-partition
                nc.scalar.activation(
                    et_t[:, ac, :], ep[:], AF.Tanh,
                    bias=tq[:, ac, b : b + 1], scale=1.0,
                )
        # v-dot: the 4 n-blocks run in distinct PE column groups, concurrent
        for ac in range(C):
            for nb in range(NBK):
                nc.tensor.matmul(
                    lgg[32 * nb : 32 * nb + 1, :],
                    vb[:, ac : ac + 1], et_ts[nb][:, ac, :],
                    start=(ac == 0), stop=(ac == C - 1),
                    tile_position=(0, 32 * nb),
                )
        # drain the whole vdot bank once; the DMA below gathers rows
        # {0,32,64,96} with a partition-strided AP into the group tile
        sbl = sm1_pool.tile([128, 512], F32, tag="sbl")
        nc.vector.tensor_copy(sbl[:], lgg[:])
        nc.sync.dma_start(lr_g[j : j + 1, :], sbl[0:128:32, :])

        if j == 3:
            b0 = 4 * g
            # masked logits, then softmax over free dim on (4, 2048)
            nc.vector.tensor_tensor(lr_g[:], lr_g[:], sup_g[:], ALU.add)
            st = sm1_pool.tile([4, 4], F32, tag="st")
            mx, nmx, smv, rc = (st[:, i : i + 1] for i in range(4))
            nc.vector.tensor_reduce(mx, lr_g[:], AX.X, ALU.max)
            nc.vector.tensor_scalar_mul(nmx, mx, -1.0)
            ex = sm1_pool.tile([4, N], F32, tag="ex")
            nc.scalar.activation(
                ex[:], lr_g[:], AF.Exp, bias=nmx, scale=1.0, accum_out=smv
            )
            nc.vector.reciprocal(rc, smv)
            nc.vector.tensor_scalar(ex[:], ex[:], rc, None, ALU.mult)

            # W^T columns for the context matmul: transpose the f32 weights
            # directly; the psum-drain copy does the bf16 cast
            wtp = ps_s.tile([128, NT, 4], F32, tag="small")
            for t in range(NT):
                nc.tensor.transpose(
                    wtp[:, t, :], ex[0:4, ts(t, 128)], identf[0:4, 0:4]
                )
            wt = sm1_pool.tile([128, NT, 4], BF16, tag="wt")
            nc.vector.tensor_copy(wt[:], wtp[:])

            # context: 4 rows run in distinct PE column groups, concurrent
            cxg = ps_s.tile([128, D], F32, tag="small")
            for t in range(NT):
                for j2 in range(4):
                    nc.tensor.matmul(
                        cxg[32 * j2 : 32 * j2 + 1, :],
                        wt[:, t, j2 : j2 + 1],
                        nb_tiles[b0 + j2][:, t, :],
                        start=(t == 0), stop=(t == NT - 1),
                        tile_position=(0, 32 * j2),
                    )
            for j2 in range(4):
                ctb = sm1_pool.tile([1, D], F32, tag="ctb")
                nc.vector.tensor_copy(ctb[:], cxg[32 * j2 : 32 * j2 + 1, :])
                nc.sync.dma_start(o_c[b0 + j2 : b0 + j2 + 1, :], ctb[:])

            nc.sync.dma_start(o_l[b0 : b0 + 4, :], lr_g[:])
            nc.sync.dma_start(o_w[b0 : b0 + 4, :], ex[:])


def _build():
    import concourse.bass as bass  # noqa: F401
    from concourse import bacc, mybir, tile

    F32 = mybir.dt.float32
    nc = bacc.Bacc("TRN2", target_bir_lowering=False, debug=False, num_devices=NCORES)
    mv = nc.declare_dram_parameter("mv", [BB, N, D], F32, isOutput=False)
    mk = nc.declare_dram_parameter("mask", [BB, N], F32, isOutput=False)
    q = nc.declare_dram_parameter("query", [BB, D], F32, isOutput=False)
    wm = nc.declare_dram_parameter("Wm", [D, D], F32, isOutput=False)
    wq = nc.declare_dram_parameter("Wq", [D, D], F32, isOutput=False)
    vv = nc.declare_dram_parameter("v", [D, 1], F32, isOutput=False)
    o_w = nc.declare_dram_parameter("weights", [BB, N], F32, isOutput=True)
    o_c = nc.declare_dram_parameter("context", [BB, D], F32, isOutput=True)
    o_l = nc.declare_dram_parameter("logits", [BB, N], F32, isOutput=True)

    from contextlib import ExitStack

    aps = (mv[:], mk[:], q[:], wm[:], wq[:], vv[:], o_w[:], o_c[:], o_l[:])
    with ExitStack() as ctx:
        tc = ctx.enter_context(tile.TileContext(nc))
        _emit(ctx, tc, nc, aps)
    nc.compile()
    return nc


def _get_nc():
    if "nc" not in _STATE:
        _STATE["nc"] = _build()
    return _STATE["nc"]


def _make_in_maps(inputs):
    mv = np.ascontiguousarray(inputs["memory_values"], dtype=np.float32)
    mk = np.ascontiguousarray(inputs["mask"], dtype=np.float32)
    q = np.ascontiguousarray(inputs["query"], dtype=np.float32)
    wm = np.ascontiguousarray(inputs["Wm"], dtype=np.float32)
    wq = np.ascontiguousarray(inputs["Wq"], dtype=np.float32)
    vv = np.ascontiguousarray(inputs["v"], dtype=np.float32)
    in_maps = []
    for c in range(NCORES):
        s = slice(c * BB, (c + 1) * BB)
        in_maps.append(
            {
                "mv": np.ascontiguousarray(mv[s]),
                "mask": np.ascontiguousarray(mk[s]),
                "query": np.ascontiguousarray(q[s]),
                "Wm": wm,
                "Wq": wq,
                "v": vv,
            }
        )
    return in_maps


def run(inputs, trace=False, **trace_kwargs):
    from concourse.bass_utils import run_bass_kernel_spmd

    nc = _get_nc()
    in_maps = _make_in_maps(inputs)
    res = run_bass_kernel_spmd(
        nc, in_maps, list(range(NCORES)), trace=trace, **trace_kwargs
    )
    outs = res.results
    weights = np.concatenate([outs[i]["weights"] for i in range(NCORES)], axis=0)
    context = np.concatenate([outs[i]["context"] for i in range(NCORES)], axis=0)
    logits = np.concatenate([outs[i]["logits"] for i in range(NCORES)], axis=0)
    return (weights, context, logits), res


def kernel(**inputs):
    (weights, context, logits), _ = run(inputs, trace=False)
    return weights, context, logits


# revision 41
# speedup vs baseline: 1.0037x; 1.0037x over previous
"""Trainium2 Bass kernel for sparse_attention (nn_Attention_171798692167).

B=128, N=2048, DM=DQ=DA=512.  Data-parallel over 8 NeuronCores: 16 batch
rows per core, Wm/Wq/v replicated.  Per row b:
    tq = query[b] @ Wq
    e = tanh(MV[b] @ Wm + tq)          (2048, 512)
    logits_raw = e @ v                 (2048,)
    logits = logits_raw + (mask-1)*1e9
    weights = softmax(logits)
    context = weights @ MV[b]          (512,)
Returns (weights, context, logits) full-shape.
"""

import sys

import numpy as np

if "/opt/trn_rl_repo" not in sys.path:
    sys.path.insert(0, "/opt/trn_rl_repo")

NCORES = 8
B, N, D = 128, 2048, 512
BB = B // NCORES  # 16 batch rows per core
C = D // 128      # 4 chunks of the 512 feature dims
NT = N // 128     # 16 n-chunks of 128
NBK = N // 512    # 4 n-blocks of 512
NEG = -1.0e9

# tuning knobs
NB_BUFS = 7
MT_BUFS = 3
ET_BUFS = 5

_STATE: dict = {}


def _emit(ctx, tc, nc, aps):
    import concourse.bass as bass
    from concourse import masks, mybir

    F32 = mybir.dt.float32
    BF16 = mybir.dt.bfloat16
    AF = mybir.ActivationFunctionType
    ALU = mybir.AluOpType
    AX = mybir.AxisListType
    ts = bass.ts

    mv, mk, q, wm, wq, vv, o_w, o_c, o_l = aps

    const_pool = ctx.enter_context(tc.tile_pool(name="const", bufs=1))
    nb_pool = ctx.enter_context(tc.tile_pool(name="nb", bufs=NB_BUFS))
    mt_pool = ctx.enter_context(tc.tile_pool(name="mt", bufs=MT_BUFS))
    et_pool = ctx.enter_context(tc.tile_pool(name="et", bufs=ET_BUFS))
    sm_pool = ctx.enter_context(tc.tile_pool(name="sm", bufs=2))
    sm1_pool = ctx.enter_context(tc.tile_pool(name="sm1", bufs=1))
    ps_t = ctx.enter_context(tc.tile_pool(name="ps_t", bufs=2, space="PSUM"))
    ps_e = ctx.enter_context(tc.tile_pool(name="ps_e", bufs=4, space="PSUM"))
    ps_v = ctx.enter_context(tc.tile_pool(name="ps_v", bufs=1, space="PSUM"))
    ps_s = ctx.enter_context(tc.tile_pool(name="ps_s", bufs=1, space="PSUM"))

    # ---- constants / params ----
    ident = const_pool.tile([128, 128], BF16, tag="ident")
    masks.make_identity(nc, ident[:])
    identf = const_pool.tile([4, 4], F32, tag="identf")
    masks.make_identity(nc, identf[:])

    # small params first so the q^T transposes can start immediately
    qb = const_pool.tile([BB, D], BF16, tag="qb")
    nc.gpsimd.dma_start(qb[:], q[:])
    vb = const_pool.tile([128, C], BF16, tag="vb")
    nc.gpsimd.dma_start(vb[:], vv[:].rearrange("(c p) o -> p (c o)", p=128))
    # Wm, Wq as (128, c, a) bf16: lhsT chunk = WmB[:, c, ts(ac,128)]
    wmb = const_pool.tile([128, C, D], BF16, tag="wmb")
    nc.gpsimd.dma_start(wmb[:], wm[:].rearrange("(c p) a -> p c a", p=128))
    # first batch row's data before Wq: the Wq-dependent tq matmuls are not
    # needed until the first tanh, but the transposes need nb[0] immediately
    nb_first = nb_pool.tile([128, NT, D], BF16, tag="nb")
    for t in range(NT):
        nc.gpsimd.dma_start(nb_first[:, t, :], mv[0][128 * t : 128 * (t + 1), :])
    wqb = const_pool.tile([128, C, D], BF16, tag="wqb")
    nc.gpsimd.dma_start(wqb[:], wq[:].rearrange("(c p) a -> p c a", p=128))
    # suppress tiles are built per group of 4 rows (mask*1e9 - 1e9, exact
    # 0 / -1e9); mask rows land at partition offset 0 via direct row DMA

    # ---- q^T then tq^T = Wq^T q^T ----
    qtp = ps_s.tile([128, C, BB], BF16, tag="small")
    for c in range(C):
        nc.tensor.transpose(qtp[:, c, :], qb[:, ts(c, 128)], ident[0:BB, 0:BB])
    qt = const_pool.tile([128, C, BB], BF16, tag="qt")
    nc.vector.tensor_copy(qt[:], qtp[:])
    tq = const_pool.tile([128, C, BB], F32, tag="tq")
    for ac in range(C):
        tqp = ps_s.tile([128, BB], F32, tag="small")
        for c in range(C):
            nc.tensor.matmul(
                tqp[:], wqb[:, c, ts(ac, 128)], qt[:, c, :],
                start=(c == 0), stop=(c == C - 1),
            )
        nc.vector.tensor_copy(tq[:, ac, :], tqp[:])

    # ---- main loop over batch rows ----
    lr_g = None
    nb_tiles = {}
    for b in range(BB):
        g, j = b // 4, b % 4

        # natural bf16 tiles: (128 n-part, t, d); SWDGE casts f32->bf16.
        # b=0 loads per-chunk so the first transposes start early.
        if b == 0:
            nb_b = nb_first
        else:
            nb_b = nb_pool.tile([128, NT, D], BF16, tag="nb")
            nc.gpsimd.dma_start(nb_b[:], mv[b].rearrange("(t p) d -> p t d", p=128))
        nb_tiles[b] = nb_b

        if j == 0:
            lr_g = sm_pool.tile([4, N], F32, tag="lr")
            sup_g = sm1_pool.tile([4, N], F32, tag="sup")
            nc.sync.dma_start(sup_g[:], mk[4 * g : 4 * g + 4, :])
            nc.vector.tensor_scalar(sup_g[:], sup_g[:], -NEG, NEG, ALU.mult, ALU.add)

        # transpose to (128 d-part, c, n); two n-chunks per psum tile so the
        # DVE drain copies are half as many and outrun the transpose stream.
        # MT is split into two half-row tiles for finer slot recycling.
        mt_h0 = mt_pool.tile([128, C, N // 2], BF16, tag="mt")
        mt_h1 = mt_pool.tile([128, C, N // 2], BF16, tag="mt")
        mt_hs = [mt_h0, mt_h1]
        lgg = ps_v.tile([128, 512], F32, tag="lg")
        for th in range(NT // 2):
            mt_h = mt_hs[th // 4]
            tp = ps_t.tile([128, C, 2, 128], BF16, tag="tp")
            for c in range(C):
                for k in range(2):
                    nc.tensor.transpose(
                        tp[:, c, k, :], nb_b[:, 2 * th + k, ts(c, 128)], ident[:]
                    )
            nc.vector.tensor_copy(
                mt_h[:, :, 256 * (th % 4) : 256 * (th % 4) + 256],
                tp[:].rearrange("p c k n -> p c (k n)"),
            )
            if th in (2, 5):
                # ~50ns normal-mode matmul: keeps the HAM clock gate warm
                # through the transpose phase (junk value, overwritten later)
                nc.tensor.matmul(
                    lgg[0:1, 0:1], vb[:, 0:1], ident[:, 0:1],
                    start=True, stop=True,
                )
        et_ts = []
        for nb in range(NBK):
            et_t = et_pool.tile([128, C, 512], BF16, tag="et")
            et_ts.append(et_t)
            for ac in range(C):
                ep = ps_e.tile([128, 512], F32, tag="pe")
                for c in range(C):
                    nc.tensor.matmul(
                        ep[:], wmb[:, c, ts(ac, 128)],
                        mt_hs[nb // 2][:, c, ts(nb % 2, 512)],
                        start=(c == 0), stop=(c == C - 1),
                    )
                # tanh(E^T + tq^T) fused on ACT; bias per-partition
                nc.scalar.activation(
                    et_t[:, ac, :], ep[:], AF.Tanh,
                    bias=tq[:, ac, b : b + 1], scale=1.0,
                )
        # v-dot: the 4 n-blocks run in distinct PE column groups, concurrent
        for ac in range(C):
            for nb in range(NBK):
                nc.tensor.matmul(
                    lgg[32 * nb : 32 * nb + 1, :],
                    vb[:, ac : ac + 1], et_ts[nb][:, ac, :],
                    start=(ac == 0), stop=(ac == C - 1),
                    tile_position=(0, 32 * nb),
                )
        # drain the whole vdot bank once; the DMA below gathers rows
        # {0,32,64,96} with a partition-strided AP into the group tile
        sbl = sm1_pool.tile([128, 512], F32, tag="sbl")
        nc.vector.tensor_copy(sbl[:], lgg[:])
        nc.sync.dma_start(lr_g[j : j + 1, :], sbl[0:128:32, :])

        if j == 3:
            b0 = 4 * g
            # masked logits, then softmax over free dim on (4, 2048)
            nc.vector.tensor_tensor(lr_g[:], lr_g[:], sup_g[:], ALU.add)
            st = sm1_pool.tile([4, 4], F32, tag="st")
            mx, nmx, smv, rc = (st[:, i : i + 1] for i in range(4))
            nc.vector.tensor_reduce(mx, lr_g[:], AX.X, ALU.max)
            nc.vector.tensor_scalar_mul(nmx, mx, -1.0)
            ex = sm1_pool.tile([4, N], F32, tag="ex")
            nc.scalar.activation(
                ex[:], lr_g[:], AF.Exp, bias=nmx, scale=1.0, accum_out=smv
            )
            nc.vector.reciprocal(rc, smv)
            nc.vector.tensor_scalar(ex[:], ex[:], rc, None, ALU.mult)

            # W^T columns for the context matmul: transpose the f32 weights
            # directly; the psum-drain copy does the bf16 cast
            wtp = ps_s.tile([128, NT, 4], F32, tag="small")
            for t in range(NT):
                nc.tensor.transpose(
                    wtp[:, t, :], ex[0:4, ts(t, 128)], identf[0:4, 0:4]
                )
            wt = sm1_pool.tile([128, NT, 4], BF16, tag="wt")
            nc.vector.tensor_copy(wt[:], wtp[:])

            # context: 4 rows run in distinct PE column groups, concurrent
            cxg = ps_s.tile([128, D], F32, tag="small")
            for t in range(NT):
                for j2 in range(4):
                    nc.tensor.matmul(
                        cxg[32 * j2 : 32 * j2 + 1, :],
                        wt[:, t, j2 : j2 + 1],
                        nb_tiles[b0 + j2][:, t, :],
                        start=(t == 0), stop=(t == NT - 1),
                        tile_position=(0, 32 * j2),
                    )
            for j2 in range(4):
                ctb = sm1_pool.tile([1, D], F32, tag="ctb")
                nc.vector.tensor_copy(ctb[:], cxg[32 * j2 : 32 * j2 + 1, :])
                nc.sync.dma_start(o_c[b0 + j2 : b0 + j2 + 1, :], ctb[:])

            nc.sync.dma_start(o_l[b0 : b0 + 4, :], lr_g[:])
            nc.sync.dma_start(o_w[b0 : b0 + 4, :], ex[:])


def _build():
    import concourse.bass as bass  # noqa: F401
    from concourse import bacc, mybir, tile

    F32 = mybir.dt.float32
    nc = bacc.Bacc("TRN2", target_bir_lowering=False, debug=False, num_devices=NCORES)
    mv = nc.declare_dram_parameter("mv", [BB, N, D], F32, isOutput=False)
    mk = nc.declare_dram_parameter("mask", [BB, N], F32, isOutput=False)
    q = nc.declare_dram_parameter("query", [BB, D], F32, isOutput=False)
    wm = nc.declare_dram_parameter("Wm", [D, D], F32, isOutput=False)
    wq = nc.declare_dram_parameter("Wq", [D, D], F32, isOutput=False)
    vv = nc.declare_dram_parameter("v", [D, 1], F32, isOutput=False)
    o_w = nc.declare_dram_parameter("weights", [BB, N], F32, isOutput=True)
    o_c = nc.declare_dram_parameter("context", [BB, D], F32, isOutput=True)
    o_l = nc.declare_dram_parameter("logits", [BB, N], F32, isOutput=True)

    from contextlib import ExitStack

    aps = (mv[:], mk[:], q[:], wm[:], wq[:], vv[:], o_w[:], o_c[:], o_l[:])
    with ExitStack() as ctx:
        tc = ctx.enter_context(tile.TileContext(nc))
        _emit(ctx, tc, nc, aps)
    nc.compile()
    return nc


def _get_nc():
    if "nc" not in _STATE:
        _STATE["nc"] = _build()
    return _STATE["nc"]


def _make_in_maps(inputs):
    mv = np.ascontiguousarray(inputs["memory_values"], dtype=np.float32)
    mk = np.ascontiguousarray(inputs["mask"], dtype=np.float32)
    q = np.ascontiguousarray(inputs["query"], dtype=np.float32)
    wm = np.ascontiguousarray(inputs["Wm"], dtype=np.float32)
    wq = np.ascontiguousarray(inputs["Wq"], dtype=np.float32)
    vv = np.ascontiguousarray(inputs["v"], dtype=np.float32)
    in_maps = []
    for c in range(NCORES):
        s = slice(c * BB, (c + 1) * BB)
        in_maps.append(
            {
                "mv": np.ascontiguousarray(mv[s]),
                "mask": np.ascontiguousarray(mk[s]),
                "query": np.ascontiguousarray(q[s]),
                "Wm": wm,
                "Wq": wq,
                "v": vv,
            }
        )
    return in_maps


def run(inputs, trace=False, **trace_kwargs):
    from concourse.bass_utils import run_bass_kernel_spmd

    nc = _get_nc()
    in_maps = _make_in_maps(inputs)
    res = run_bass_kernel_spmd(
        nc, in_maps, list(range(NCORES)), trace=trace, **trace_kwargs
    )
    outs = res.results
    weights = np.concatenate([outs[i]["weights"] for i in range(NCORES)], axis=0)
    context = np.concatenate([outs[i]["context"] for i in range(NCORES)], axis=0)
    logits = np.concatenate([outs[i]["logits"] for i in range(NCORES)], axis=0)
    return (weights, context, logits), res


def kernel(**inputs):
    (weights, context, logits), _ = run(inputs, trace=False)
    return weights, context, logits


# revision 42
# speedup vs baseline: 1.0053x; 1.0017x over previous
"""Trainium2 Bass kernel for sparse_attention (nn_Attention_171798692167).

B=128, N=2048, DM=DQ=DA=512.  Data-parallel over 8 NeuronCores: 16 batch
rows per core, Wm/Wq/v replicated.  Per row b:
    tq = query[b] @ Wq
    e = tanh(MV[b] @ Wm + tq)          (2048, 512)
    logits_raw = e @ v                 (2048,)
    logits = logits_raw + (mask-1)*1e9
    weights = softmax(logits)
    context = weights @ MV[b]          (512,)
Returns (weights, context, logits) full-shape.
"""

import sys

import numpy as np

if "/opt/trn_rl_repo" not in sys.path:
    sys.path.insert(0, "/opt/trn_rl_repo")

NCORES = 8
B, N, D = 128, 2048, 512
BB = B // NCORES  # 16 batch rows per core
C = D // 128      # 4 chunks of the 512 feature dims
NT = N // 128     # 16 n-chunks of 128
NBK = N // 512    # 4 n-blocks of 512
NEG = -1.0e9

# tuning knobs
NB_BUFS = 7
MT_BUFS = 3
ET_BUFS = 5

_STATE: dict = {}


def _emit(ctx, tc, nc, aps):
    import concourse.bass as bass
    from concourse import masks, mybir

    F32 = mybir.dt.float32
    BF16 = mybir.dt.bfloat16
    AF = mybir.ActivationFunctionType
    ALU = mybir.AluOpType
    AX = mybir.AxisListType
    ts = bass.ts

    mv, mk, q, wm, wq, vv, o_w, o_c, o_l = aps

    const_pool = ctx.enter_context(tc.tile_pool(name="const", bufs=1))
    nb_pool = ctx.enter_context(tc.tile_pool(name="nb", bufs=NB_BUFS))
    mt_pool = ctx.enter_context(tc.tile_pool(name="mt", bufs=MT_BUFS))
    et_pool = ctx.enter_context(tc.tile_pool(name="et", bufs=ET_BUFS))
    sm_pool = ctx.enter_context(tc.tile_pool(name="sm", bufs=2))
    sm1_pool = ctx.enter_context(tc.tile_pool(name="sm1", bufs=1))
    ps_t = ctx.enter_context(tc.tile_pool(name="ps_t", bufs=2, space="PSUM"))
    ps_e = ctx.enter_context(tc.tile_pool(name="ps_e", bufs=4, space="PSUM"))
    ps_v = ctx.enter_context(tc.tile_pool(name="ps_v", bufs=1, space="PSUM"))
    ps_s = ctx.enter_context(tc.tile_pool(name="ps_s", bufs=1, space="PSUM"))

    # ---- constants / params ----
    ident = const_pool.tile([128, 128], BF16, tag="ident")
    masks.make_identity(nc, ident[:])
    identf = const_pool.tile([4, 4], F32, tag="identf")
    masks.make_identity(nc, identf[:])

    # small params first so the q^T transposes can start immediately
    qb = const_pool.tile([BB, D], BF16, tag="qb")
    nc.gpsimd.dma_start(qb[:], q[:])
    vb = const_pool.tile([128, C], BF16, tag="vb")
    nc.gpsimd.dma_start(vb[:], vv[:].rearrange("(c p) o -> p (c o)", p=128))
    # Wm, Wq as (128, c, a) bf16: lhsT chunk = WmB[:, c, ts(ac,128)]
    wmb = const_pool.tile([128, C, D], BF16, tag="wmb")
    nc.gpsimd.dma_start(wmb[:], wm[:].rearrange("(c p) a -> p c a", p=128))
    # first batch row's data before Wq: the Wq-dependent tq matmuls are not
    # needed until the first tanh, but the transposes need nb[0] immediately
    nb_first = nb_pool.tile([128, NT, D], BF16, tag="nb")
    for t in range(NT):
        nc.gpsimd.dma_start(nb_first[:, t, :], mv[0][128 * t : 128 * (t + 1), :])
    wqb = const_pool.tile([128, C, D], BF16, tag="wqb")
    nc.gpsimd.dma_start(wqb[:], wq[:].rearrange("(c p) a -> p c a", p=128))
    # suppress tiles are built per group of 4 rows (mask*1e9 - 1e9, exact
    # 0 / -1e9); mask rows land at partition offset 0 via direct row DMA

    # ---- q^T then tq^T = Wq^T q^T ----
    qtp = ps_s.tile([128, C, BB], BF16, tag="small")
    for c in range(C):
        nc.tensor.transpose(qtp[:, c, :], qb[:, ts(c, 128)], ident[0:BB, 0:BB])
    qt = const_pool.tile([128, C, BB], BF16, tag="qt")
    nc.vector.tensor_copy(qt[:], qtp[:])
    tq = const_pool.tile([128, C, BB], F32, tag="tq")
    for ac in range(C):
        tqp = ps_s.tile([128, BB], F32, tag="small")
        for c in range(C):
            nc.tensor.matmul(
                tqp[:], wqb[:, c, ts(ac, 128)], qt[:, c, :],
                start=(c == 0), stop=(c == C - 1),
            )
        nc.vector.tensor_copy(tq[:, ac, :], tqp[:])

    # ---- main loop over batch rows ----
    lr_g = None
    nb_tiles = {}
    for b in range(BB):
        g, j = b // 4, b % 4

        # natural bf16 tiles: (128 n-part, t, d); SWDGE casts f32->bf16.
        # b=0 loads per-chunk so the first transposes start early.
        if b == 0:
            nb_b = nb_first
        else:
            nb_b = nb_pool.tile([128, NT, D], BF16, tag="nb")
            nc.gpsimd.dma_start(nb_b[:], mv[b].rearrange("(t p) d -> p t d", p=128))
        nb_tiles[b] = nb_b

        if j == 0:
            lr_g = sm_pool.tile([4, N], F32, tag="lr")
            sup_g = sm1_pool.tile([4, N], F32, tag="sup")
            nc.sync.dma_start(sup_g[:], mk[4 * g : 4 * g + 4, :])
            nc.vector.tensor_scalar(sup_g[:], sup_g[:], -NEG, NEG, ALU.mult, ALU.add)

        # transpose to (128 d-part, c, n); two n-chunks per psum tile so the
        # DVE drain copies are half as many and outrun the transpose stream.
        # MT is split into two half-row tiles for finer slot recycling.
        mt_h0 = mt_pool.tile([128, C, N // 2], BF16, tag="mt")
        mt_h1 = mt_pool.tile([128, C, N // 2], BF16, tag="mt")
        mt_hs = [mt_h0, mt_h1]
        lgg = ps_v.tile([128, 512], F32, tag="lg")
        for th in range(NT // 2):
            mt_h = mt_hs[th // 4]
            tp = ps_t.tile([128, C, 2, 128], BF16, tag="tp")
            for c in range(C):
                for k in range(2):
                    nc.tensor.transpose(
                        tp[:, c, k, :], nb_b[:, 2 * th + k, ts(c, 128)], ident[:]
                    )
            # alternate psum-drain copies between DVE and ACT so the
            # transpose phase is not bound by one engine's copy rate
            cp_engine = nc.vector.tensor_copy if th % 2 == 0 else nc.scalar.copy
            cp_engine(
                mt_h[:, :, 256 * (th % 4) : 256 * (th % 4) + 256],
                tp[:].rearrange("p c k n -> p c (k n)"),
            )
            if th in (2, 5):
                # ~50ns normal-mode matmul: keeps the HAM clock gate warm
                # through the transpose phase (junk value, overwritten later)
                nc.tensor.matmul(
                    lgg[0:1, 0:1], vb[:, 0:1], ident[:, 0:1],
                    start=True, stop=True,
                )
        et_ts = []
        for nb in range(NBK):
            et_t = et_pool.tile([128, C, 512], BF16, tag="et")
            et_ts.append(et_t)
            for ac in range(C):
                ep = ps_e.tile([128, 512], F32, tag="pe")
                for c in range(C):
                    nc.tensor.matmul(
                        ep[:], wmb[:, c, ts(ac, 128)],
                        mt_hs[nb // 2][:, c, ts(nb % 2, 512)],
                        start=(c == 0), stop=(c == C - 1),
                    )
                # tanh(E^T + tq^T) fused on ACT; bias per-partition
                nc.scalar.activation(
                    et_t[:, ac, :], ep[:], AF.Tanh,
                    bias=tq[:, ac, b : b + 1], scale=1.0,
                )
        # v-dot: the 4 n-blocks run in distinct PE column groups, concurrent
        for ac in range(C):
            for nb in range(NBK):
                nc.tensor.matmul(
                    lgg[32 * nb : 32 * nb + 1, :],
                    vb[:, ac : ac + 1], et_ts[nb][:, ac, :],
                    start=(ac == 0), stop=(ac == C - 1),
                    tile_position=(0, 32 * nb),
                )
        # drain the whole vdot bank once; the DMA below gathers rows
        # {0,32,64,96} with a partition-strided AP into the group tile
        sbl = sm1_pool.tile([128, 512], F32, tag="sbl")
        nc.vector.tensor_copy(sbl[:], lgg[:])
        nc.sync.dma_start(lr_g[j : j + 1, :], sbl[0:128:32, :])

        if j == 3:
            b0 = 4 * g
            # masked logits, then softmax over free dim on (4, 2048)
            nc.vector.tensor_tensor(lr_g[:], lr_g[:], sup_g[:], ALU.add)
            st = sm1_pool.tile([4, 4], F32, tag="st")
            mx, nmx, smv, rc = (st[:, i : i + 1] for i in range(4))
            nc.vector.tensor_reduce(mx, lr_g[:], AX.X, ALU.max)
            nc.vector.tensor_scalar_mul(nmx, mx, -1.0)
            ex = sm1_pool.tile([4, N], F32, tag="ex")
            nc.scalar.activation(
                ex[:], lr_g[:], AF.Exp, bias=nmx, scale=1.0, accum_out=smv
            )
            nc.vector.reciprocal(rc, smv)
            nc.vector.tensor_scalar(ex[:], ex[:], rc, None, ALU.mult)

            # W^T columns for the context matmul: transpose the f32 weights
            # directly; the psum-drain copy does the bf16 cast
            wtp = ps_s.tile([128, NT, 4], F32, tag="small")
            for t in range(NT):
                nc.tensor.transpose(
                    wtp[:, t, :], ex[0:4, ts(t, 128)], identf[0:4, 0:4]
                )
            wt = sm1_pool.tile([128, NT, 4], BF16, tag="wt")
            nc.vector.tensor_copy(wt[:], wtp[:])

            # context: 4 rows run in distinct PE column groups, concurrent
            cxg = ps_s.tile([128, D], F32, tag="small")
            for t in range(NT):
                for j2 in range(4):
                    nc.tensor.matmul(
                        cxg[32 * j2 : 32 * j2 + 1, :],
                        wt[:, t, j2 : j2 + 1],
                        nb_tiles[b0 + j2][:, t, :],
                        start=(t == 0), stop=(t == NT - 1),
                        tile_position=(0, 32 * j2),
                    )
            for j2 in range(4):
                ctb = sm1_pool.tile([1, D], F32, tag="ctb")
                nc.vector.tensor_copy(ctb[:], cxg[32 * j2 : 32 * j2 + 1, :])
                nc.sync.dma_start(o_c[b0 + j2 : b0 + j2 + 1, :], ctb[:])

            nc.sync.dma_start(o_l[b0 : b0 + 4, :], lr_g[:])
            nc.sync.dma_start(o_w[b0 : b0 + 4, :], ex[:])


def _build():
    import concourse.bass as bass  # noqa: F401
    from concourse import bacc, mybir, tile

    F32 = mybir.dt.float32
    nc = bacc.Bacc("TRN2", target_bir_lowering=False, debug=False, num_devices=NCORES)
    mv = nc.declare_dram_parameter("mv", [BB, N, D], F32, isOutput=False)
    mk = nc.declare_dram_parameter("mask", [BB, N], F32, isOutput=False)
    q = nc.declare_dram_parameter("query", [BB, D], F32, isOutput=False)
    wm = nc.declare_dram_parameter("Wm", [D, D], F32, isOutput=False)
    wq = nc.declare_dram_parameter("Wq", [D, D], F32, isOutput=False)
    vv = nc.declare_dram_parameter("v", [D, 1], F32, isOutput=False)
    o_w = nc.declare_dram_parameter("weights", [BB, N], F32, isOutput=True)
    o_c = nc.declare_dram_parameter("context", [BB, D], F32, isOutput=True)
    o_l = nc.declare_dram_parameter("logits", [BB, N], F32, isOutput=True)

    from contextlib import ExitStack

    aps = (mv[:], mk[:], q[:], wm[:], wq[:], vv[:], o_w[:], o_c[:], o_l[:])
    with ExitStack() as ctx:
        tc = ctx.enter_context(tile.TileContext(nc))
        _emit(ctx, tc, nc, aps)
    nc.compile()
    return nc


def _get_nc():
    if "nc" not in _STATE:
        _STATE["nc"] = _build()
    return _STATE["nc"]


def _make_in_maps(inputs):
    mv = np.ascontiguousarray(inputs["memory_values"], dtype=np.float32)
    mk = np.ascontiguousarray(inputs["mask"], dtype=np.float32)
    q = np.ascontiguousarray(inputs["query"], dtype=np.float32)
    wm = np.ascontiguousarray(inputs["Wm"], dtype=np.float32)
    wq = np.ascontiguousarray(inputs["Wq"], dtype=np.float32)
    vv = np.ascontiguousarray(inputs["v"], dtype=np.float32)
    in_maps = []
    for c in range(NCORES):
        s = slice(c * BB, (c + 1) * BB)
        in_maps.append(
            {
                "mv": np.ascontiguousarray(mv[s]),
                "mask": np.ascontiguousarray(mk[s]),
                "query": np.ascontiguousarray(q[s]),
                "Wm": wm,
                "Wq": wq,
                "v": vv,
            }
        )
    return in_maps


def run(inputs, trace=False, **trace_kwargs):
    from concourse.bass_utils import run_bass_kernel_spmd

    nc = _get_nc()
    in_maps = _make_in_maps(inputs)
    res = run_bass_kernel_spmd(
        nc, in_maps, list(range(NCORES)), trace=trace, **trace_kwargs
    )
    outs = res.results
    weights = np.concatenate([outs[i]["weights"] for i in range(NCORES)], axis=0)
    context = np.concatenate([outs[i]["context"] for i in range(NCORES)], axis=0)
    logits = np.concatenate([outs[i]["logits"] for i in range(NCORES)], axis=0)
    return (weights, context, logits), res


def kernel(**inputs):
    (weights, context, logits), _ = run(inputs, trace=False)
    return weights, context, logits


# revision 43
# speedup vs baseline: 1.0808x; 1.0751x over previous
"""Trainium2 Bass kernel for sparse_attention (nn_Attention_171798692167).

B=128, N=2048, DM=DQ=DA=512.  Data-parallel over 8 NeuronCores: 16 batch
rows per core, Wm/Wq/v replicated.  Per row b:
    tq = query[b] @ Wq
    e = tanh(MV[b] @ Wm + tq)          (2048, 512)
    logits_raw = e @ v                 (2048,)
    logits = logits_raw + (mask-1)*1e9
    weights = softmax(logits)
    context = weights @ MV[b]          (512,)
Returns (weights, context, logits) full-shape.
"""

import sys

import numpy as np

if "/opt/trn_rl_repo" not in sys.path:
    sys.path.insert(0, "/opt/trn_rl_repo")

NCORES = 8
B, N, D = 128, 2048, 512
BB = B // NCORES  # 16 batch rows per core
C = D // 128      # 4 chunks of the 512 feature dims
NT = N // 128     # 16 n-chunks of 128
NBK = N // 512    # 4 n-blocks of 512
NEG = -1.0e9

# tuning knobs
NB_BUFS = 7
MT_BUFS = 3
ET_BUFS = 5

_STATE: dict = {}


def _emit(ctx, tc, nc, aps):
    import concourse.bass as bass
    from concourse import masks, mybir

    F32 = mybir.dt.float32
    BF16 = mybir.dt.bfloat16
    AF = mybir.ActivationFunctionType
    ALU = mybir.AluOpType
    AX = mybir.AxisListType
    ts = bass.ts

    mv, mk, q, wm, wq, vv, o_w, o_c, o_l = aps

    const_pool = ctx.enter_context(tc.tile_pool(name="const", bufs=1))
    nb_pool = ctx.enter_context(tc.tile_pool(name="nb", bufs=NB_BUFS))
    mt_pool = ctx.enter_context(tc.tile_pool(name="mt", bufs=MT_BUFS))
    et_pool = ctx.enter_context(tc.tile_pool(name="et", bufs=ET_BUFS))
    sm_pool = ctx.enter_context(tc.tile_pool(name="sm", bufs=2))
    sm1_pool = ctx.enter_context(tc.tile_pool(name="sm1", bufs=1))
    ps_t = ctx.enter_context(tc.tile_pool(name="ps_t", bufs=2, space="PSUM"))
    ps_e = ctx.enter_context(tc.tile_pool(name="ps_e", bufs=4, space="PSUM"))
    ps_v = ctx.enter_context(tc.tile_pool(name="ps_v", bufs=1, space="PSUM"))
    ps_s = ctx.enter_context(tc.tile_pool(name="ps_s", bufs=1, space="PSUM"))

    # ---- constants / params ----
    ident = const_pool.tile([128, 128], BF16, tag="ident")
    masks.make_identity(nc, ident[:])
    identf = const_pool.tile([4, 4], F32, tag="identf")
    masks.make_identity(nc, identf[:])

    # small params first so the q^T transposes can start immediately
    qb = const_pool.tile([BB, D], BF16, tag="qb")
    nc.gpsimd.dma_start(qb[:], q[:])
    vb = const_pool.tile([128, C], BF16, tag="vb")
    nc.gpsimd.dma_start(vb[:], vv[:].rearrange("(c p) o -> p (c o)", p=128))
    # Wm, Wq as (128, c, a) bf16: lhsT chunk = WmB[:, c, ts(ac,128)]
    wmb = const_pool.tile([128, C, D], BF16, tag="wmb")
    nc.gpsimd.dma_start(wmb[:], wm[:].rearrange("(c p) a -> p c a", p=128))
    # first batch row's data before Wq: the Wq-dependent tq matmuls are not
    # needed until the first tanh, but the transposes need nb[0] immediately
    nb_first = nb_pool.tile([128, NT, D], BF16, tag="nb")
    for t in range(NT):
        nc.gpsimd.dma_start(nb_first[:, t, :], mv[0][128 * t : 128 * (t + 1), :])
    wqb = const_pool.tile([128, C, D], BF16, tag="wqb")
    nc.gpsimd.dma_start(wqb[:], wq[:].rearrange("(c p) a -> p c a", p=128))
    # suppress tiles are built per group of 4 rows (mask*1e9 - 1e9, exact
    # 0 / -1e9); mask rows land at partition offset 0 via direct row DMA

    # ---- q^T then tq^T = Wq^T q^T ----
    qtp = ps_s.tile([128, C, BB], BF16, tag="small")
    for c in range(C):
        nc.tensor.transpose(qtp[:, c, :], qb[:, ts(c, 128)], ident[0:BB, 0:BB])
    qt = const_pool.tile([128, C, BB], BF16, tag="qt")
    nc.vector.tensor_copy(qt[:], qtp[:])
    tq = const_pool.tile([128, C, BB], F32, tag="tq")
    for ac in range(C):
        tqp = ps_s.tile([128, BB], F32, tag="small")
        for c in range(C):
            nc.tensor.matmul(
                tqp[:], wqb[:, c, ts(ac, 128)], qt[:, c, :],
                start=(c == 0), stop=(c == C - 1),
            )
        nc.vector.tensor_copy(tq[:, ac, :], tqp[:])

    # ---- main loop over batch rows ----
    lr_g = None
    nb_tiles = {}
    for b in range(BB):
        g, j = b // 4, b % 4

        # natural bf16 tiles: (128 n-part, t, d); SWDGE casts f32->bf16.
        # b=0 loads per-chunk so the first transposes start early.
        if b == 0:
            nb_b = nb_first
        else:
            nb_b = nb_pool.tile([128, NT, D], BF16, tag="nb")
            nc.gpsimd.dma_start(nb_b[:], mv[b].rearrange("(t p) d -> p t d", p=128))
        nb_tiles[b] = nb_b

        if j == 0:
            lr_g = sm_pool.tile([4, N], F32, tag="lr")
        # this row's suppress values spread to partitions {0,32,64,96}
        # (matching the vdot bank layout) so masking fuses into the drain
        ssp = sm1_pool.tile([128, 512], F32, tag="ssp")
        nc.sync.dma_start(ssp[0:128:32, :], mk[b : b + 1, :])
        nc.vector.tensor_scalar(ssp[:], ssp[:], -NEG, NEG, ALU.mult, ALU.add)

        # transpose to (128 d-part, c, n); two n-chunks per psum tile so the
        # DVE drain copies are half as many and outrun the transpose stream.
        # MT is split into two half-row tiles for finer slot recycling.
        mt_h0 = mt_pool.tile([128, C, N // 2], BF16, tag="mt")
        mt_h1 = mt_pool.tile([128, C, N // 2], BF16, tag="mt")
        mt_hs = [mt_h0, mt_h1]
        lgg = ps_v.tile([128, 512], F32, tag="lg")
        for th in range(NT // 2):
            mt_h = mt_hs[th // 4]
            tp = ps_t.tile([128, C, 2, 128], BF16, tag="tp")
            for c in range(C):
                for k in range(2):
                    nc.tensor.transpose(
                        tp[:, c, k, :], nb_b[:, 2 * th + k, ts(c, 128)], ident[:]
                    )
            # alternate psum-drain copies between DVE and ACT so the
            # transpose phase is not bound by one engine's copy rate
            cp_engine = nc.vector.tensor_copy if th % 2 == 0 else nc.scalar.copy
            cp_engine(
                mt_h[:, :, 256 * (th % 4) : 256 * (th % 4) + 256],
                tp[:].rearrange("p c k n -> p c (k n)"),
            )
            if th in (2, 5):
                # ~50ns normal-mode matmul: keeps the HAM clock gate warm
                # through the transpose phase (junk value, overwritten later)
                nc.tensor.matmul(
                    lgg[0:1, 0:1], vb[:, 0:1], ident[:, 0:1],
                    start=True, stop=True,
                )
        et_ts = []
        for nb in range(NBK):
            et_t = et_pool.tile([128, C, 512], BF16, tag="et")
            et_ts.append(et_t)
            for ac in range(C):
                ep = ps_e.tile([128, 512], F32, tag="pe")
                for c in range(C):
                    nc.tensor.matmul(
                        ep[:], wmb[:, c, ts(ac, 128)],
                        mt_hs[nb // 2][:, c, ts(nb % 2, 512)],
                        start=(c == 0), stop=(c == C - 1),
                    )
                # tanh(E^T + tq^T) fused on ACT; bias per-partition
                nc.scalar.activation(
                    et_t[:, ac, :], ep[:], AF.Tanh,
                    bias=tq[:, ac, b : b + 1], scale=1.0,
                )
        # v-dot: the 4 n-blocks run in distinct PE column groups, concurrent
        for ac in range(C):
            for nb in range(NBK):
                nc.tensor.matmul(
                    lgg[32 * nb : 32 * nb + 1, :],
                    vb[:, ac : ac + 1], et_ts[nb][:, ac, :],
                    start=(ac == 0), stop=(ac == C - 1),
                    tile_position=(0, 32 * nb),
                )
        # drain the whole vdot bank once, adding the suppress values in the
        # same pass; the DMA gathers rows {0,32,64,96} with a partition-
        # strided AP into the group tile (rows arrive already masked)
        sbl = sm1_pool.tile([128, 512], F32, tag="sbl")
        nc.vector.tensor_tensor(sbl[:], lgg[:], ssp[:], ALU.add)
        nc.sync.dma_start(lr_g[j : j + 1, :], sbl[0:128:32, :])

        if j == 3:
            b0 = 4 * g
            # rows already masked; softmax over free dim on (4, 2048)
            st = sm1_pool.tile([4, 4], F32, tag="st")
            mx, nmx, smv, rc = (st[:, i : i + 1] for i in range(4))
            nc.vector.tensor_reduce(mx, lr_g[:], AX.X, ALU.max)
            nc.vector.tensor_scalar_mul(nmx, mx, -1.0)
            ex = sm1_pool.tile([4, N], F32, tag="ex")
            nc.scalar.activation(
                ex[:], lr_g[:], AF.Exp, bias=nmx, scale=1.0, accum_out=smv
            )
            nc.vector.reciprocal(rc, smv)
            nc.vector.tensor_scalar(ex[:], ex[:], rc, None, ALU.mult)

            # W^T columns for the context matmul: transpose the f32 weights
            # directly; the psum-drain copy does the bf16 cast
            wtp = ps_s.tile([128, NT, 4], F32, tag="small")
            for t in range(NT):
                nc.tensor.transpose(
                    wtp[:, t, :], ex[0:4, ts(t, 128)], identf[0:4, 0:4]
                )
            wt = sm1_pool.tile([128, NT, 4], BF16, tag="wt")
            nc.vector.tensor_copy(wt[:], wtp[:])

            # context: 4 rows run in distinct PE column groups, concurrent
            cxg = ps_s.tile([128, D], F32, tag="small")
            for t in range(NT):
                for j2 in range(4):
                    nc.tensor.matmul(
                        cxg[32 * j2 : 32 * j2 + 1, :],
                        wt[:, t, j2 : j2 + 1],
                        nb_tiles[b0 + j2][:, t, :],
                        start=(t == 0), stop=(t == NT - 1),
                        tile_position=(0, 32 * j2),
                    )
            sbc = sm1_pool.tile([128, D], F32, tag="sbc")
            nc.vector.tensor_copy(sbc[:], cxg[:])
            nc.sync.dma_start(o_c[b0 : b0 + 4, :], sbc[0:128:32, :])

            nc.sync.dma_start(o_l[b0 : b0 + 4, :], lr_g[:])
            nc.sync.dma_start(o_w[b0 : b0 + 4, :], ex[:])


def _build():
    import concourse.bass as bass  # noqa: F401
    from concourse import bacc, mybir, tile

    F32 = mybir.dt.float32
    nc = bacc.Bacc("TRN2", target_bir_lowering=False, debug=False, num_devices=NCORES)
    mv = nc.declare_dram_parameter("mv", [BB, N, D], F32, isOutput=False)
    mk = nc.declare_dram_parameter("mask", [BB, N], F32, isOutput=False)
    q = nc.declare_dram_parameter("query", [BB, D], F32, isOutput=False)
    wm = nc.declare_dram_parameter("Wm", [D, D], F32, isOutput=False)
    wq = nc.declare_dram_parameter("Wq", [D, D], F32, isOutput=False)
    vv = nc.declare_dram_parameter("v", [D, 1], F32, isOutput=False)
    o_w = nc.declare_dram_parameter("weights", [BB, N], F32, isOutput=True)
    o_c = nc.declare_dram_parameter("context", [BB, D], F32, isOutput=True)
    o_l = nc.declare_dram_parameter("logits", [BB, N], F32, isOutput=True)

    from contextlib import ExitStack

    aps = (mv[:], mk[:], q[:], wm[:], wq[:], vv[:], o_w[:], o_c[:], o_l[:])
    with ExitStack() as ctx:
        tc = ctx.enter_context(tile.TileContext(nc))
        _emit(ctx, tc, nc, aps)
    nc.compile()
    return nc


def _get_nc():
    if "nc" not in _STATE:
        _STATE["nc"] = _build()
    return _STATE["nc"]


def _make_in_maps(inputs):
    mv = np.ascontiguousarray(inputs["memory_values"], dtype=np.float32)
    mk = np.ascontiguousarray(inputs["mask"], dtype=np.float32)
    q = np.ascontiguousarray(inputs["query"], dtype=np.float32)
    wm = np.ascontiguousarray(inputs["Wm"], dtype=np.float32)
    wq = np.ascontiguousarray(inputs["Wq"], dtype=np.float32)
    vv = np.ascontiguousarray(inputs["v"], dtype=np.float32)
    in_maps = []
    for c in range(NCORES):
        s = slice(c * BB, (c + 1) * BB)
        in_maps.append(
            {
                "mv": np.ascontiguousarray(mv[s]),
                "mask": np.ascontiguousarray(mk[s]),
                "query": np.ascontiguousarray(q[s]),
                "Wm": wm,
                "Wq": wq,
                "v": vv,
            }
        )
    return in_maps


def run(inputs, trace=False, **trace_kwargs):
    from concourse.bass_utils import run_bass_kernel_spmd

    nc = _get_nc()
    in_maps = _make_in_maps(inputs)
    res = run_bass_kernel_spmd(
        nc, in_maps, list(range(NCORES)), trace=trace, **trace_kwargs
    )
    outs = res.results
    weights = np.concatenate([outs[i]["weights"] for i in range(NCORES)], axis=0)
    context = np.concatenate([outs[i]["context"] for i in range(NCORES)], axis=0)
    logits = np.concatenate([outs[i]["logits"] for i in range(NCORES)], axis=0)
    return (weights, context, logits), res


def kernel(**inputs):
    (weights, context, logits), _ = run(inputs, trace=False)
    return weights, context, logits


# revision 44
# speedup vs baseline: 1.0903x; 1.0087x over previous
"""Trainium2 Bass kernel for sparse_attention (nn_Attention_171798692167).

B=128, N=2048, DM=DQ=DA=512.  Data-parallel over 8 NeuronCores: 16 batch
rows per core, Wm/Wq/v replicated.  Per row b:
    tq = query[b] @ Wq
    e = tanh(MV[b] @ Wm + tq)          (2048, 512)
    logits_raw = e @ v                 (2048,)
    logits = logits_raw + (mask-1)*1e9
    weights = softmax(logits)
    context = weights @ MV[b]          (512,)
Returns (weights, context, logits) full-shape.
"""

import sys

import numpy as np

if "/opt/trn_rl_repo" not in sys.path:
    sys.path.insert(0, "/opt/trn_rl_repo")

NCORES = 8
B, N, D = 128, 2048, 512
BB = B // NCORES  # 16 batch rows per core
C = D // 128      # 4 chunks of the 512 feature dims
NT = N // 128     # 16 n-chunks of 128
NBK = N // 512    # 4 n-blocks of 512
NEG = -1.0e9

# tuning knobs
NB_BUFS = 7
MT_BUFS = 3
ET_BUFS = 5

_STATE: dict = {}


def _emit(ctx, tc, nc, aps):
    import concourse.bass as bass
    from concourse import masks, mybir

    F32 = mybir.dt.float32
    BF16 = mybir.dt.bfloat16
    AF = mybir.ActivationFunctionType
    ALU = mybir.AluOpType
    AX = mybir.AxisListType
    ts = bass.ts

    mv, mk, q, wm, wq, vv, o_w, o_c, o_l = aps

    const_pool = ctx.enter_context(tc.tile_pool(name="const", bufs=1))
    nb_pool = ctx.enter_context(tc.tile_pool(name="nb", bufs=NB_BUFS))
    mt_pool = ctx.enter_context(tc.tile_pool(name="mt", bufs=MT_BUFS))
    et_pool = ctx.enter_context(tc.tile_pool(name="et", bufs=ET_BUFS))
    sm_pool = ctx.enter_context(tc.tile_pool(name="sm", bufs=2))
    sm1_pool = ctx.enter_context(tc.tile_pool(name="sm1", bufs=1))
    ps_t = ctx.enter_context(tc.tile_pool(name="ps_t", bufs=2, space="PSUM"))
    ps_e = ctx.enter_context(tc.tile_pool(name="ps_e", bufs=4, space="PSUM"))
    ps_v = ctx.enter_context(tc.tile_pool(name="ps_v", bufs=1, space="PSUM"))
    ps_s = ctx.enter_context(tc.tile_pool(name="ps_s", bufs=1, space="PSUM"))

    # ---- constants / params ----
    ident = const_pool.tile([128, 128], BF16, tag="ident")
    masks.make_identity(nc, ident[:])
    identf = const_pool.tile([4, 4], F32, tag="identf")
    masks.make_identity(nc, identf[:])

    # small params first so the q^T transposes can start immediately
    qb = const_pool.tile([BB, D], BF16, tag="qb")
    nc.gpsimd.dma_start(qb[:], q[:])
    vb = const_pool.tile([128, C], BF16, tag="vb")
    nc.gpsimd.dma_start(vb[:], vv[:].rearrange("(c p) o -> p (c o)", p=128))
    # Wm, Wq as (128, c, a) bf16: lhsT chunk = WmB[:, c, ts(ac,128)]
    wmb = const_pool.tile([128, C, D], BF16, tag="wmb")
    nc.gpsimd.dma_start(wmb[:], wm[:].rearrange("(c p) a -> p c a", p=128))
    # first batch row's data before Wq: the Wq-dependent tq matmuls are not
    # needed until the first tanh, but the transposes need nb[0] immediately
    nb_first = nb_pool.tile([128, NT, D], BF16, tag="nb")
    for t in range(NT):
        nc.gpsimd.dma_start(nb_first[:, t, :], mv[0][128 * t : 128 * (t + 1), :])
    wqb = const_pool.tile([128, C, D], BF16, tag="wqb")
    nc.gpsimd.dma_start(wqb[:], wq[:].rearrange("(c p) a -> p c a", p=128))
    # suppress tiles are built per group of 4 rows (mask*1e9 - 1e9, exact
    # 0 / -1e9); mask rows land at partition offset 0 via direct row DMA

    # ---- q^T then tq^T = Wq^T q^T ----
    qtp = ps_s.tile([128, C, BB], BF16, tag="small")
    for c in range(C):
        nc.tensor.transpose(qtp[:, c, :], qb[:, ts(c, 128)], ident[0:BB, 0:BB])
    qt = const_pool.tile([128, C, BB], BF16, tag="qt")
    nc.vector.tensor_copy(qt[:], qtp[:])
    tq = const_pool.tile([128, C, BB], F32, tag="tq")
    for ac in range(C):
        tqp = ps_s.tile([128, BB], F32, tag="small")
        for c in range(C):
            nc.tensor.matmul(
                tqp[:], wqb[:, c, ts(ac, 128)], qt[:, c, :],
                start=(c == 0), stop=(c == C - 1),
            )
        nc.vector.tensor_copy(tq[:, ac, :], tqp[:])

    # ---- main loop over batch rows ----
    lr_g = None
    nb_tiles = {}
    for b in range(BB):
        g, j = b // 4, b % 4

        # natural bf16 tiles: (128 n-part, t, d); SWDGE casts f32->bf16.
        # b=0 loads per-chunk so the first transposes start early.
        if b == 0:
            nb_b = nb_first
        else:
            nb_b = nb_pool.tile([128, NT, D], BF16, tag="nb")
            nc.gpsimd.dma_start(nb_b[:], mv[b].rearrange("(t p) d -> p t d", p=128))
        nb_tiles[b] = nb_b

        if j == 0:
            lr_g = sm_pool.tile([4, N], F32, tag="lr")
        # this row's suppress values spread to partitions {0,32,64,96}
        # (matching the vdot bank layout) so masking fuses into the drain
        ssp = sm1_pool.tile([128, 512], F32, tag="ssp")
        nc.sync.dma_start(ssp[0:128:32, :], mk[b : b + 1, :])
        nc.vector.tensor_scalar(ssp[:], ssp[:], -NEG, NEG, ALU.mult, ALU.add)

        # transpose to (128 d-part, c, n); two n-chunks per psum tile so the
        # DVE drain copies are half as many and outrun the transpose stream.
        # MT is split into two half-row tiles for finer slot recycling.
        mt_h0 = mt_pool.tile([128, C, N // 2], BF16, tag="mt")
        mt_h1 = mt_pool.tile([128, C, N // 2], BF16, tag="mt")
        mt_hs = [mt_h0, mt_h1]
        lgg = ps_v.tile([128, 512], F32, tag="lg")
        for th in range(NT // 2):
            mt_h = mt_hs[th // 4]
            tp = ps_t.tile([128, C, 2, 128], BF16, tag="tp")
            for c in range(C):
                for k in range(2):
                    nc.tensor.transpose(
                        tp[:, c, k, :], nb_b[:, 2 * th + k, ts(c, 128)], ident[:]
                    )
            nc.vector.tensor_copy(
                mt_h[:, :, 256 * (th % 4) : 256 * (th % 4) + 256],
                tp[:].rearrange("p c k n -> p c (k n)"),
            )
            if th in (2, 5):
                # ~50ns normal-mode matmul: keeps the HAM clock gate warm
                # through the transpose phase (junk value, overwritten later)
                nc.tensor.matmul(
                    lgg[0:1, 0:1], vb[:, 0:1], ident[:, 0:1],
                    start=True, stop=True,
                )
        et_ts = []
        for nb in range(NBK):
            et_t = et_pool.tile([128, C, 512], BF16, tag="et")
            et_ts.append(et_t)
            for ac in range(C):
                ep = ps_e.tile([128, 512], F32, tag="pe")
                for c in range(C):
                    nc.tensor.matmul(
                        ep[:], wmb[:, c, ts(ac, 128)],
                        mt_hs[nb // 2][:, c, ts(nb % 2, 512)],
                        start=(c == 0), stop=(c == C - 1),
                    )
                # tanh(E^T + tq^T) fused on ACT; bias per-partition
                nc.scalar.activation(
                    et_t[:, ac, :], ep[:], AF.Tanh,
                    bias=tq[:, ac, b : b + 1], scale=1.0,
                )
        # v-dot: the 4 n-blocks run in distinct PE column groups, concurrent
        for ac in range(C):
            for nb in range(NBK):
                nc.tensor.matmul(
                    lgg[32 * nb : 32 * nb + 1, :],
                    vb[:, ac : ac + 1], et_ts[nb][:, ac, :],
                    start=(ac == 0), stop=(ac == C - 1),
                    tile_position=(0, 32 * nb),
                )
        # drain the whole vdot bank once, adding the suppress values in the
        # same pass; the DMA gathers rows {0,32,64,96} with a partition-
        # strided AP into the group tile (rows arrive already masked)
        sbl = sm1_pool.tile([128, 512], F32, tag="sbl")
        nc.vector.tensor_tensor(sbl[:], lgg[:], ssp[:], ALU.add)
        nc.sync.dma_start(lr_g[j : j + 1, :], sbl[0:128:32, :])

        if j == 3:
            b0 = 4 * g
            # rows already masked; softmax over free dim on (4, 2048)
            st = sm1_pool.tile([4, 4], F32, tag="st")
            mx, nmx, smv, rc = (st[:, i : i + 1] for i in range(4))
            nc.vector.tensor_reduce(mx, lr_g[:], AX.X, ALU.max)
            nc.vector.tensor_scalar_mul(nmx, mx, -1.0)
            ex = sm1_pool.tile([4, N], F32, tag="ex")
            nc.scalar.activation(
                ex[:], lr_g[:], AF.Exp, bias=nmx, scale=1.0, accum_out=smv
            )
            nc.vector.reciprocal(rc, smv)
            nc.vector.tensor_scalar(ex[:], ex[:], rc, None, ALU.mult)

            # W^T columns for the context matmul: transpose the f32 weights
            # directly; the psum-drain copy does the bf16 cast
            wtp = ps_s.tile([128, NT, 4], F32, tag="small")
            for t in range(NT):
                nc.tensor.transpose(
                    wtp[:, t, :], ex[0:4, ts(t, 128)], identf[0:4, 0:4]
                )
            wt = sm1_pool.tile([128, NT, 4], BF16, tag="wt")
            nc.vector.tensor_copy(wt[:], wtp[:])

            # context: 4 rows run in distinct PE column groups, concurrent
            cxg = ps_s.tile([128, D], F32, tag="small")
            for t in range(NT):
                for j2 in range(4):
                    nc.tensor.matmul(
                        cxg[32 * j2 : 32 * j2 + 1, :],
                        wt[:, t, j2 : j2 + 1],
                        nb_tiles[b0 + j2][:, t, :],
                        start=(t == 0), stop=(t == NT - 1),
                        tile_position=(0, 32 * j2),
                    )
            sbc = sm1_pool.tile([128, D], F32, tag="sbc")
            nc.vector.tensor_copy(sbc[:], cxg[:])
            nc.sync.dma_start(o_c[b0 : b0 + 4, :], sbc[0:128:32, :])

            nc.sync.dma_start(o_l[b0 : b0 + 4, :], lr_g[:])
            nc.sync.dma_start(o_w[b0 : b0 + 4, :], ex[:])


def _build():
    import concourse.bass as bass  # noqa: F401
    from concourse import bacc, mybir, tile

    F32 = mybir.dt.float32
    nc = bacc.Bacc("TRN2", target_bir_lowering=False, debug=False, num_devices=NCORES)
    mv = nc.declare_dram_parameter("mv", [BB, N, D], F32, isOutput=False)
    mk = nc.declare_dram_parameter("mask", [BB, N], F32, isOutput=False)
    q = nc.declare_dram_parameter("query", [BB, D], F32, isOutput=False)
    wm = nc.declare_dram_parameter("Wm", [D, D], F32, isOutput=False)
    wq = nc.declare_dram_parameter("Wq", [D, D], F32, isOutput=False)
    vv = nc.declare_dram_parameter("v", [D, 1], F32, isOutput=False)
    o_w = nc.declare_dram_parameter("weights", [BB, N], F32, isOutput=True)
    o_c = nc.declare_dram_parameter("context", [BB, D], F32, isOutput=True)
    o_l = nc.declare_dram_parameter("logits", [BB, N], F32, isOutput=True)

    from contextlib import ExitStack

    aps = (mv[:], mk[:], q[:], wm[:], wq[:], vv[:], o_w[:], o_c[:], o_l[:])
    with ExitStack() as ctx:
        tc = ctx.enter_context(tile.TileContext(nc))
        _emit(ctx, tc, nc, aps)
    nc.compile()
    return nc


def _get_nc():
    if "nc" not in _STATE:
        _STATE["nc"] = _build()
    return _STATE["nc"]


def _make_in_maps(inputs):
    mv = np.ascontiguousarray(inputs["memory_values"], dtype=np.float32)
    mk = np.ascontiguousarray(inputs["mask"], dtype=np.float32)
    q = np.ascontiguousarray(inputs["query"], dtype=np.float32)
    wm = np.ascontiguousarray(inputs["Wm"], dtype=np.float32)
    wq = np.ascontiguousarray(inputs["Wq"], dtype=np.float32)
    vv = np.ascontiguousarray(inputs["v"], dtype=np.float32)
    in_maps = []
    for c in range(NCORES):
        s = slice(c * BB, (c + 1) * BB)
        in_maps.append(
            {
                "mv": np.ascontiguousarray(mv[s]),
                "mask": np.ascontiguousarray(mk[s]),
                "query": np.ascontiguousarray(q[s]),
                "Wm": wm,
                "Wq": wq,
                "v": vv,
            }
        )
    return in_maps


def run(inputs, trace=False, **trace_kwargs):
    from concourse.bass_utils import run_bass_kernel_spmd

    nc = _get_nc()
    in_maps = _make_in_maps(inputs)
    res = run_bass_kernel_spmd(
        nc, in_maps, list(range(NCORES)), trace=trace, **trace_kwargs
    )
    outs = res.results
    weights = np.concatenate([outs[i]["weights"] for i in range(NCORES)], axis=0)
    context = np.concatenate([outs[i]["context"] for i in range(NCORES)], axis=0)
    logits = np.concatenate([outs[i]["logits"] for i in range(NCORES)], axis=0)
    return (weights, context, logits), res


def kernel(**inputs):
    (weights, context, logits), _ = run(inputs, trace=False)
    return weights, context, logits
